# revision 37
# baseline (speedup 1.0000x reference)
"""Multi-head causal attention (B=2, S=2048, D=2048, H=16) on 8 trn2 cores.

Sharding: tensor-parallel over heads. Core c owns heads 2c, 2c+1 (256
features of q/k/v). Each core computes its heads' QKV projections (+RoPE),
causal attention, and a partial output through its slice of wo. The 8
partial outputs are summed on the host (the "all-reduce").

All matmul operands are bf16 (fp32 PSUM accumulate); IO tensors are bf16
(halves HBM traffic, enables FWL weight loads). f32r and bf16 both stream
1 cycle/row on the PE, so precision is the only trade (measured ~3e-3 rel).

Per-core layouts:
  qT, kT: [hd=128 partitions, head, token] bf16. Head dims permuted (evens
          then odds) via host-permuted wq/wk rows so RoPE pairs sit in
          partition halves.
  v:      natural [token, feature] bf16, computed directly with x chunks as
          the stationary operand (no PE transposes).
  scoresT chunk = matmul(lhsT=kT chunk, rhs=qT tile) -> [kt 128, q<=512] PSUM
  probsT  = exp(scoresT * scale) on ACT -> bf16 SBUF (scores O(1), no max)
  denom   = running bf16 chunk-sum on DVE, then gpsimd partition_all_reduce
            (broadcast to 128 partitions), reciprocal_approx_fast on DVE.
            (No PE matmuls or PSUM banks burned on the softmax denominator.)
  attnT accum = matmul(lhsT=v chunk, rhs=probsT) -> [hd, q] PSUM, then
            normalized into aT bf16 at eviction.
  out rows = matmul(lhsT=aT t-sub, rhs=woT) -> [t 128, j 512] PSUM, staged
            into a [128, 4, 2048] bf16 tile, one 2MB DMA per (b, qtile).

QKV runs as six sequential single-PSUM-bank passes per token tile
(q0,q1,k0,k1,v...), which keeps the whole kernel inside 8 PSUM banks:
2 (qkv ping-pong) + 2 (scores) + 2 (attn accum) + 2 (wo out).

RoPE per eviction tile qp [128, 512] (top rows even dims xr, bottom odd xi):
  qraw = copy(qp)->bf16  (ACT; frees the PSUM bank immediately)
  qsw  = [xi; xr]        (2 ACT half-copies)
  qT   = qraw*[c;c] - qsw*[s;-s]   (3 DVE bf16 ops)
"""

import math

import numpy as np

B = 2
S = 2048
D = 2048
H = 16
HD = 128
NCORES = 8
FPC = D // NCORES          # 256 features (2 heads) per core
P = 128
ND = D // P                # 16 contraction chunks
TT = 512                   # token tile (matmul free dim)
NTT = S // TT              # 4 token tiles per batch
NKT = S // P               # 16 key chunks per batch
SCALE = 1.0 / math.sqrt(HD)

_CACHE = {}


def _build_nc():
    import concourse.bass as bass  # noqa: F401
    from concourse import bacc
    import concourse.bass_isa as bass_isa
    import concourse.mybir as mybir
    import concourse.tile as tile

    f32 = mybir.dt.float32
    bf16 = mybir.dt.bfloat16
    MUL = mybir.AluOpType.mult
    SUB = mybir.AluOpType.subtract
    ADD = mybir.AluOpType.add
    EXP = mybir.ActivationFunctionType.Exp

    nc = bacc.Bacc(None, target_bir_lowering=False)

    # All inputs are host-pre-tiled to [128 partitions, ...contiguous] so
    # every DMA is 128 large contiguous descriptors (line-rate), not
    # thousands of 512B strided ones.
    NGT = B * NTT  # 8 global token tiles
    xTb = nc.dram_tensor("xTb", [P, NGT, ND, TT], bf16, kind="ExternalInput")
    wqT = nc.dram_tensor("wqT", [P, ND, FPC], bf16, kind="ExternalInput")
    wkT = nc.dram_tensor("wkT", [P, ND, FPC], bf16, kind="ExternalInput")
    wvT = nc.dram_tensor("wvT", [P, ND, FPC], bf16, kind="ExternalInput")
    woT = nc.dram_tensor("woT", [P, 2, D], bf16, kind="ExternalInput")
    cosS = nc.dram_tensor("cosS", [P, S], bf16, kind="ExternalInput")
    sinS = nc.dram_tensor("sinS", [P, S], bf16, kind="ExternalInput")  # [s; -s]
    masks = nc.dram_tensor("masks", [P, 2 * P], bf16, kind="ExternalInput")
    onesd = nc.dram_tensor("onesd", [P, P], bf16, kind="ExternalInput")
    outp = nc.dram_tensor("outp", [B * S, D], bf16, kind="ExternalOutput")

    outr = outp.rearrange("(r p) d -> p r d", p=P)     # [128, 32, D]

    with tile.TileContext(nc) as tc:
        with (
            tc.tile_pool(name="res", bufs=1) as res,
            tc.tile_pool(name="xp", bufs=2) as xp,
            tc.tile_pool(name="ropep", bufs=2) as ropep,
            tc.tile_pool(name="probsp", bufs=6) as probsp,
            tc.tile_pool(name="accp", bufs=2) as accp,
            tc.tile_pool(name="dp", bufs=2) as dp,
            tc.tile_pool(name="aTp", bufs=2) as aTp,
            tc.tile_pool(name="ostp", bufs=2) as ostp,
            # qkv-pass tiles, score tiles, and denominator tiles share one
            # 4-slot PSUM pool (same tag): pure-attention phases get 4 score
            # buffers, QKV phases use 2 + scores 2.
            tc.tile_pool(name="ps_qs", bufs=4, space="PSUM") as ps_qs,
            tc.tile_pool(name="ps_a", bufs=2, space="PSUM") as ps_a,
            tc.tile_pool(name="ps_o", bufs=2, space="PSUM") as ps_o,
        ):
            # resident tensors (qT/kT/v double-buffered by batch to avoid
            # cross-batch WAR stalls between attention(b) and QKV(b+1))
            wq_sb = res.tile([P, ND, FPC], bf16)
            wk_sb = res.tile([P, ND, FPC], bf16)
            wv_sb = res.tile([P, ND, FPC], bf16)
            wo_sb = res.tile([P, 2, D], bf16)
            cos_sb = res.tile([P, S], bf16)
            sin_sb = res.tile([P, S], bf16)
            mask_sb = res.tile([P, 2 * P], bf16)
            ones_sb = res.tile([P, P], bf16)
            qT_sb = res.tile([P, B, 2, S], bf16)
            kT_sb = res.tile([P, B, 2, S], bf16)
            v_sb = res.tile([P, B, NKT, FPC], bf16)

            # weight/constant loads on the scalar HWDGE queue (ACT is idle at
            # start; separate ring from sync so they stream concurrently with
            # the x-tile loads). Ordered by first use; wq split so the first
            # q-pass can start after 512KB.
            nc.scalar.dma_start(out=wq_sb[:, 0:4, :], in_=wqT[:, 0:4, :])
            nc.scalar.dma_start(out=wq_sb[:, 4:16, :], in_=wqT[:, 4:16, :])
            nc.scalar.dma_start(out=wk_sb[:, 0:4, :], in_=wkT[:, 0:4, :])
            nc.scalar.dma_start(out=wk_sb[:, 4:16, :], in_=wkT[:, 4:16, :])
            nc.scalar.dma_start(out=wv_sb[:, 0:4, :], in_=wvT[:, 0:4, :])
            nc.scalar.dma_start(out=wv_sb[:, 4:16, :], in_=wvT[:, 4:16, :])
            nc.scalar.dma_start(out=cos_sb[:], in_=cosS[:])
            nc.scalar.dma_start(out=sin_sb[:], in_=sinS[:])
            nc.scalar.dma_start(out=mask_sb[:], in_=masks[:])
            nc.scalar.dma_start(out=ones_sb[:], in_=onesd[:])
            nc.scalar.dma_start(out=wo_sb[:], in_=woT[:])

            # HAM warm-up: ~10 dummy matmuls on zeroed SBUF run during the
            # initial weight/x DMA wait, so the first real matmuls execute at
            # 2.4GHz (K=8/8) instead of cold 1.2GHz.
            warm_sb = res.tile([P, TT], bf16)
            nc.vector.memset(warm_sb[:], 0)
            w_ps = ps_o.tile([P, TT], f32, name="ops")
            for _ in range(10):
                nc.tensor.matmul(
                    w_ps[:], warm_sb[:, 0:P], warm_sb[:],
                    start=True, stop=True)

            wo_jobs = []
            ost_state = {}  # id(ost) -> [count, b, qt]

            def emit_wo_group(b, qt, aT, ts, jc, ost):
                o_ps = ps_o.tile([P, TT], f32, name="ops")
                for h in range(2):
                    nc.tensor.matmul(
                        o_ps[:],
                        aT[:, h, ts * P:(ts + 1) * P],
                        wo_sb[:, h, jc * TT:(jc + 1) * TT],
                        start=(h == 0), stop=(h == 1),
                    )
                nc.any.tensor_copy(ost[:, ts, jc * TT:(jc + 1) * TT], o_ps[:])
                st = ost_state[id(ost)]
                st[0] += 1
                # jobs run ts-major: store each 512-token-row slab as soon as
                # its 4 column groups land (shrinks the kernel tail)
                if st[0] % 4 == 0:
                    tsd = st[0] // 4 - 1
                    r0 = (b * S + qt * TT) // P + tsd
                    nc.sync.dma_start(
                        out=outr[:, r0:r0 + 1, :], in_=ost[:, tsd:tsd + 1, :])

            def pop_wo(n=1):
                for _ in range(n):
                    if wo_jobs:
                        emit_wo_group(*wo_jobs.pop(0))

            def rope_evict(ps, dst, fc, tsl):
                qraw = ropep.tile([P, TT], bf16, name="qraw")
                nc.scalar.copy(qraw[:], ps[:])
                qsw = ropep.tile([P, TT], bf16, name="qsw")
                nc.scalar.copy(qsw[0:64, :], qraw[64:128, :])
                nc.scalar.copy(qsw[64:128, :], qraw[0:64, :])
                p1 = ropep.tile([P, TT], bf16, name="p1")
                nc.vector.tensor_tensor(
                    out=p1[:], in0=qraw[:], in1=cos_sb[:, tsl], op=MUL)
                p2 = ropep.tile([P, TT], bf16, name="p2")
                nc.vector.tensor_tensor(
                    out=p2[:], in0=qsw[:], in1=sin_sb[:, tsl], op=MUL)
                nc.vector.tensor_tensor(
                    out=dst[:, fc, tsl], in0=p1[:], in1=p2[:], op=SUB)

            # x-tile prefetch: DMAs ride the sync HWDGE FIFO, but each tile's
            # dma_start is emitted one step ahead of its consumer so it never
            # queues behind the late-firing output stores popped in between.
            xt_tiles = {}

            def prefetch_x(b, tt):
                gt = b * NTT + tt
                xt = xp.tile([P, ND, TT], bf16, name="xt")
                if b == 0 and tt == 0:
                    # loaded in 4 slices so the first q-pass starts at 512KB
                    for g in range(4):
                        nc.sync.dma_start(
                            out=xt[:, 4 * g:4 * g + 4, :],
                            in_=xTb[:, gt, 4 * g:4 * g + 4, :])
                else:
                    nc.sync.dma_start(out=xt[:], in_=xTb[:, gt, :, :])
                xt_tiles[(b, tt)] = xt

            def emit_qkv(b, tt):
                tsl = slice(tt * TT, (tt + 1) * TT)
                # issue the NEXT tile's x load now (runs during this tile) —
                # except on the very first tile, where it would steal DMA
                # bandwidth from the critical wq/wk/wv loads; there it is
                # deferred to the v-pass (still ~15us ahead of its use).
                nxt = (b, tt + 1) if tt + 1 < NTT else (b + 1, 0)
                first = (b == 0 and tt == 0)
                if nxt[0] < B and not first:
                    prefetch_x(*nxt)
                xt = xt_tiles.pop((b, tt))

                # q/k passes: one PSUM bank per (which, fc), sequential
                for w_sb, dst in ((wq_sb, qT_sb), (wk_sb, kT_sb)):
                    for fc in range(2):
                        ps = ps_qs.tile([P, TT], f32, name="qs")
                        for d in range(ND):
                            nc.tensor.matmul(
                                ps[:],
                                w_sb[:, d, fc * P:(fc + 1) * P],
                                xt[:, d, :],
                                start=(d == 0), stop=(d == ND - 1),
                            )
                        rope_evict(ps, dst[:, b], fc, tsl)
                        pop_wo()
                if first:
                    prefetch_x(*nxt)
                # v pass: natural layout, one 128-token sub-chunk per bank
                for sub in range(4):
                    ps = ps_qs.tile([P, TT], f32, name="qs")
                    for d in range(ND):
                        nc.tensor.matmul(
                            ps[:, 0:FPC],
                            xt[:, d, sub * P:(sub + 1) * P],
                            wv_sb[:, d, :],
                            start=(d == 0), stop=(d == ND - 1),
                        )
                    nc.any.tensor_copy(
                        v_sb[:, b, tt * 4 + sub, :], ps[:, 0:FPC])
                    pop_wo()

            def emit_attention(b, qt):
                t0g = b * S
                nkt = 4 * qt + 4
                aT = aTp.tile([P, 2, TT], bf16, name="aT")
                ost = ostp.tile([P, 4, D], bf16, name="ost")
                ost_state[id(ost)] = [0, b, qt]
                # spread pending wo jobs over this tile's pair iterations
                niter = nkt
                wo_quota = len(wo_jobs)
                emitted = [0]

                def pace(i):
                    want = wo_quota * (i + 1) // niter
                    while emitted[0] < want and wo_jobs:
                        emit_wo_group(*wo_jobs.pop(0))
                        emitted[0] += 1

                it = 0
                for h in range(2):
                    acc = accp.tile([P, TT], bf16, name="acc")
                    a_ps = ps_a.tile([P, TT], f32, name="aps")
                    for kt in range(nkt):
                        if kt % 2 == 0:
                            pace(it); it += 1
                        o = kt - 4 * qt
                        c0 = 0 if o < 0 else o * P
                        csl = slice(c0, TT)
                        s_ps = ps_qs.tile([P, TT], f32, name="qs")
                        nc.tensor.matmul(
                            s_ps[:, csl],
                            kT_sb[:, b, h, kt * P:(kt + 1) * P],
                            qT_sb[:, b, h, qt * TT + c0:(qt + 1) * TT],
                            start=True, stop=True,
                        )
                        pr = probsp.tile([P, TT], bf16, name="probs")
                        nc.scalar.activation(
                            pr[:, csl], s_ps[:, csl], EXP, scale=SCALE)
                        if o >= 0:
                            nc.vector.tensor_tensor(
                                out=pr[:, c0:c0 + P], in0=pr[:, c0:c0 + P],
                                in1=mask_sb[:, P:2 * P], op=MUL)
                        # denominator chunk-sums on the (mostly idle) GpSimd
                        if kt == 0:
                            nc.gpsimd.tensor_copy(acc[:], pr[:])
                        else:
                            nc.gpsimd.tensor_tensor(
                                out=acc[:, csl], in0=acc[:, csl],
                                in1=pr[:, csl], op=ADD)
                        nc.tensor.matmul(
                            a_ps[:, csl],
                            v_sb[:, b, kt, h * P:(h + 1) * P],
                            pr[:, csl],
                            start=(kt == 0), stop=(kt == nkt - 1),
                        )
                    # softmax denominator: ones-matmul broadcasts the
                    # partition-sum of acc to all 128 partitions in one
                    # 512-cycle PE op; ~1.5us chain latency to aT.
                    d_ps = ps_qs.tile([P, TT], f32, name="qs")
                    nc.tensor.matmul(
                        d_ps[:], ones_sb[:], acc[:], start=True, stop=True)
                    rb = dp.tile([P, TT], f32, name="rb")
                    nc.vector.reciprocal_approx_fast(rb[:], d_ps[:])
                    nc.vector.tensor_tensor(
                        out=aT[:, h, :], in0=a_ps[:], in1=rb[:], op=MUL)
                for ts in range(4):
                    for jc in range(D // TT):
                        wo_jobs.append((b, qt, aT, ts, jc, ost))

            # schedule: attention lags QKV by one tile within each batch
            prefetch_x(0, 0)
            for b in range(B):
                emit_qkv(b, 0)
                for tt in range(1, NTT):
                    emit_qkv(b, tt)
                    emit_attention(b, tt - 1)
                emit_attention(b, NTT - 1)
            pop_wo(len(wo_jobs))
    nc.compile()
    return nc


def _host_prep(x, wq, wk, wv, wo):
    import ml_dtypes

    bf = ml_dtypes.bfloat16
    x = np.asarray(x, dtype=np.float32)
    wq = np.asarray(wq, dtype=np.float32)
    wk = np.asarray(wk, dtype=np.float32)
    wv = np.asarray(wv, dtype=np.float32)
    wo = np.asarray(wo, dtype=np.float32)

    # x pre-tiled: [P, global token tile, d-chunk, token] with contiguous
    # per-partition runs per (tile, d-chunk)
    xT = x.reshape(B * S, D).T                        # [D, B*S]
    xTt = np.ascontiguousarray(
        xT.reshape(ND, P, B * NTT, TT).transpose(1, 2, 0, 3)).astype(bf)

    def tile_w(w):  # [D, FPC] -> [P, ND, FPC]
        return np.ascontiguousarray(
            w.reshape(ND, P, FPC).transpose(1, 0, 2)).astype(bf)

    # permute q/k head dims: per head, even dims then odd dims
    perm = np.concatenate(
        [h * HD + np.concatenate([np.arange(0, HD, 2), np.arange(1, HD, 2)])
         for h in range(H)]
    )
    wq_p = wq[perm]
    wk_p = wk[perm]

    # rope tables; cos stacked twice, sin stacked [s; -s]
    inv_freq = 1.0 / (10000.0 ** (np.arange(0, HD, 2, dtype=np.float64) / HD))
    t = np.arange(S, dtype=np.float64)
    freqs = t[:, None] * inv_freq[None, :]            # [S, 64]
    cosT = np.cos(freqs).T.astype(np.float32)         # [64, S]
    sinT = np.sin(freqs).T.astype(np.float32)
    cosS = np.ascontiguousarray(np.vstack([cosT, cosT])).astype(bf)
    sinS = np.ascontiguousarray(np.vstack([sinT, -sinT])).astype(bf)

    # masks: [zeros(128) | lower-triangular(128)] for the diagonal blocks
    pidx = np.arange(P)[:, None]
    qidx = np.arange(P)[None, :]
    tri = (qidx >= pidx).astype(np.float32)
    m = np.ascontiguousarray(
        np.hstack([np.zeros((P, P), np.float32), tri])).astype(bf)
    ones = np.ones((P, P), dtype=np.float32).astype(bf)

    in_maps = []
    for c in range(NCORES):
        fs = slice(c * FPC, (c + 1) * FPC)
        woc = wo[:, fs].T                              # [256, D]
        in_maps.append({
            "xTb": xTt,
            "wqT": tile_w(wq_p[fs].T),                 # [P, ND, FPC]
            "wkT": tile_w(wk_p[fs].T),
            "wvT": tile_w(wv[fs].T),
            "woT": np.ascontiguousarray(
                woc.reshape(2, P, D).transpose(1, 0, 2)).astype(bf),
            "cosS": cosS,
            "sinS": sinS,
            "masks": m,
            "onesd": ones,
        })
    return in_maps


def _run(inputs, trace=False):
    from concourse.bass_utils import run_bass_kernel_spmd

    if "nc" not in _CACHE:
        _CACHE["nc"] = _build_nc()
    nc = _CACHE["nc"]

    in_maps = _host_prep(
        inputs["x"], inputs["wq"], inputs["wk"], inputs["wv"], inputs["wo"]
    )
    res = run_bass_kernel_spmd(nc, in_maps, list(range(NCORES)), trace=trace)
    acc = None
    for c in range(NCORES):
        part = res.results[c]["outp"].astype(np.float32)
        acc = part if acc is None else acc + part
    out = acc.reshape(B, S, D).astype(np.float32)
    return out, res


def kernel(**inputs) -> np.ndarray:
    out, _ = _run(inputs, trace=False)
    return out


# revision 39
# speedup vs baseline: 1.3254x; 1.3254x over previous
"""Multi-head causal attention (B=2, S=2048, D=2048, H=16) on 8 trn2 cores.

Sharding: tensor-parallel over heads. Core c owns heads 2c, 2c+1 (256
features of q/k/v). Each core computes its heads' QKV projections (+RoPE),
causal attention, and a partial output through its slice of wo. The 8
partial outputs are summed on the host (the "all-reduce").

All matmul operands are bf16 (fp32 PSUM accumulate); IO tensors are bf16
(halves HBM traffic, enables FWL weight loads). f32r and bf16 both stream
1 cycle/row on the PE, so precision is the only trade (measured ~3e-3 rel).

Per-core layouts:
  qT, kT: [hd=128 partitions, head, token] bf16. Head dims permuted (evens
          then odds) via host-permuted wq/wk rows so RoPE pairs sit in
          partition halves.
  v:      natural [token, feature] bf16, computed directly with x chunks as
          the stationary operand (no PE transposes).
  scoresT chunk = matmul(lhsT=kT chunk, rhs=qT tile) -> [kt 128, q<=512] PSUM
  probsT  = exp(scoresT * scale) on ACT -> bf16 SBUF (scores O(1), no max)
  denom   = running bf16 chunk-sum on DVE, then gpsimd partition_all_reduce
            (broadcast to 128 partitions), reciprocal_approx_fast on DVE.
            (No PE matmuls or PSUM banks burned on the softmax denominator.)
  attnT accum = matmul(lhsT=v chunk, rhs=probsT) -> [hd, q] PSUM, then
            normalized into aT bf16 at eviction.
  out rows = matmul(lhsT=aT t-sub, rhs=woT) -> [t 128, j 512] PSUM, staged
            into a [128, 4, 2048] bf16 tile, one 2MB DMA per (b, qtile).

QKV runs as six sequential single-PSUM-bank passes per token tile
(q0,q1,k0,k1,v...), which keeps the whole kernel inside 8 PSUM banks:
2 (qkv ping-pong) + 2 (scores) + 2 (attn accum) + 2 (wo out).

RoPE per eviction tile qp [128, 512] (top rows even dims xr, bottom odd xi):
  qraw = copy(qp)->bf16  (ACT; frees the PSUM bank immediately)
  qsw  = [xi; xr]        (2 ACT half-copies)
  qT   = qraw*[c;c] - qsw*[s;-s]   (3 DVE bf16 ops)
"""

import math

import numpy as np

B = 2
S = 2048
D = 2048
H = 16
HD = 128
NCORES = 8
FPC = D // NCORES          # 256 features (2 heads) per core
P = 128
ND = D // P                # 16 contraction chunks
TT = 512                   # token tile (matmul free dim)
NTT = S // TT              # 4 token tiles per batch
NKT = S // P               # 16 key chunks per batch
SCALE = 1.0 / math.sqrt(HD)

_CACHE = {}


def _build_nc():
    import concourse.bass as bass  # noqa: F401
    from concourse import bacc
    import concourse.bass_isa as bass_isa
    import concourse.mybir as mybir
    import concourse.tile as tile

    f32 = mybir.dt.float32
    bf16 = mybir.dt.bfloat16
    MUL = mybir.AluOpType.mult
    SUB = mybir.AluOpType.subtract
    ADD = mybir.AluOpType.add
    EXP = mybir.ActivationFunctionType.Exp

    nc = bacc.Bacc(None, target_bir_lowering=False)

    # All inputs are host-pre-tiled to [128 partitions, ...contiguous] so
    # every DMA is 128 large contiguous descriptors (line-rate), not
    # thousands of 512B strided ones.
    NGT = B * NTT  # 8 global token tiles
    xTb = nc.dram_tensor("xTb", [P, NGT, ND, TT], bf16, kind="ExternalInput")
    wqT = nc.dram_tensor("wqT", [P, ND, FPC], bf16, kind="ExternalInput")
    wkT = nc.dram_tensor("wkT", [P, ND, FPC], bf16, kind="ExternalInput")
    wvT = nc.dram_tensor("wvT", [P, ND, FPC], bf16, kind="ExternalInput")
    woT = nc.dram_tensor("woT", [P, 2, D], bf16, kind="ExternalInput")
    cosS = nc.dram_tensor("cosS", [P, S], bf16, kind="ExternalInput")
    sinS = nc.dram_tensor("sinS", [P, S], bf16, kind="ExternalInput")  # [s; -s]
    masks = nc.dram_tensor("masks", [P, 2 * P], bf16, kind="ExternalInput")
    onesd = nc.dram_tensor("onesd", [P, P], bf16, kind="ExternalInput")
    outp = nc.dram_tensor("outp", [B * S, D], bf16, kind="ExternalOutput")

    outr = outp.rearrange("(r p) d -> p r d", p=P)     # [128, 32, D]

    with tile.TileContext(nc) as tc:
        with (
            tc.tile_pool(name="res", bufs=1) as res,
            tc.tile_pool(name="xp", bufs=2) as xp,
            tc.tile_pool(name="ropep", bufs=2) as ropep,
            tc.tile_pool(name="probsp", bufs=6) as probsp,
            tc.tile_pool(name="accp", bufs=2) as accp,
            tc.tile_pool(name="dp", bufs=2) as dp,
            tc.tile_pool(name="aTp", bufs=2) as aTp,
            tc.tile_pool(name="ostp", bufs=2) as ostp,
            # qkv-pass tiles, score tiles, and denominator tiles share one
            # 4-slot PSUM pool (same tag): pure-attention phases get 4 score
            # buffers, QKV phases use 2 + scores 2.
            tc.tile_pool(name="ps_qs", bufs=4, space="PSUM") as ps_qs,
            tc.tile_pool(name="ps_a", bufs=2, space="PSUM") as ps_a,
            tc.tile_pool(name="ps_o", bufs=2, space="PSUM") as ps_o,
        ):
            # resident tensors (qT/kT/v double-buffered by batch to avoid
            # cross-batch WAR stalls between attention(b) and QKV(b+1))
            wq_sb = res.tile([P, ND, FPC], bf16)
            wk_sb = res.tile([P, ND, FPC], bf16)
            wv_sb = res.tile([P, ND, FPC], bf16)
            wo_sb = res.tile([P, 2, D], bf16)
            cos_sb = res.tile([P, S], bf16)
            sin_sb = res.tile([P, S], bf16)
            mask_sb = res.tile([P, 2 * P], bf16)
            ones_sb = res.tile([P, P], bf16)
            qT_sb = res.tile([P, B, 2, S], bf16)
            kT_sb = res.tile([P, B, 2, S], bf16)
            v_sb = res.tile([P, B, NKT, FPC], bf16)

            # weight/constant loads on the scalar HWDGE queue (ACT is idle at
            # start; separate ring from sync so they stream concurrently with
            # the x-tile loads). Ordered by first use; wq split so the first
            # q-pass can start after 512KB.
            nc.scalar.dma_start(out=wq_sb[:, 0:4, :], in_=wqT[:, 0:4, :])
            nc.scalar.dma_start(out=wq_sb[:, 4:16, :], in_=wqT[:, 4:16, :])
            nc.scalar.dma_start(out=wk_sb[:, 0:4, :], in_=wkT[:, 0:4, :])
            nc.scalar.dma_start(out=wk_sb[:, 4:16, :], in_=wkT[:, 4:16, :])
            nc.scalar.dma_start(out=wv_sb[:, 0:4, :], in_=wvT[:, 0:4, :])
            nc.scalar.dma_start(out=wv_sb[:, 4:16, :], in_=wvT[:, 4:16, :])
            nc.scalar.dma_start(out=cos_sb[:], in_=cosS[:])
            nc.scalar.dma_start(out=sin_sb[:], in_=sinS[:])
            nc.scalar.dma_start(out=mask_sb[:], in_=masks[:])
            nc.scalar.dma_start(out=ones_sb[:], in_=onesd[:])
            nc.scalar.dma_start(out=wo_sb[:], in_=woT[:])

            # HAM warm-up: ~10 dummy matmuls on zeroed SBUF run during the
            # initial weight/x DMA wait, so the first real matmuls execute at
            # 2.4GHz (K=8/8) instead of cold 1.2GHz.
            warm_sb = res.tile([P, TT], bf16)
            nc.vector.memset(warm_sb[:], 0)
            w_ps = ps_o.tile([P, TT], f32, name="ops")
            for _ in range(10):
                nc.tensor.matmul(
                    w_ps[:], warm_sb[:, 0:P], warm_sb[:],
                    start=True, stop=True)

            wo_jobs = []
            ost_state = {}  # id(ost) -> [count, b, qt]

            def emit_wo_group(b, qt, aT, ts, jc, ost):
                o_ps = ps_o.tile([P, TT], f32, name="ops")
                for h in range(2):
                    nc.tensor.matmul(
                        o_ps[:],
                        aT[:, h, ts * P:(ts + 1) * P],
                        wo_sb[:, h, jc * TT:(jc + 1) * TT],
                        start=(h == 0), stop=(h == 1),
                    )
                nc.any.tensor_copy(ost[:, ts, jc * TT:(jc + 1) * TT], o_ps[:])
                st = ost_state[id(ost)]
                st[0] += 1
                # jobs run ts-major: store each 512-token-row slab as soon as
                # its 4 column groups land (shrinks the kernel tail)
                if st[0] % 4 == 0:
                    tsd = st[0] // 4 - 1
                    r0 = (b * S + qt * TT) // P + tsd
                    nc.sync.dma_start(
                        out=outr[:, r0:r0 + 1, :], in_=ost[:, tsd:tsd + 1, :])

            def pop_wo(n=1):
                for _ in range(n):
                    if wo_jobs:
                        emit_wo_group(*wo_jobs.pop(0))

            def rope_evict(ps, dst, fc, tsl):
                qraw = ropep.tile([P, TT], bf16, name="qraw")
                nc.scalar.copy(qraw[:], ps[:])
                qsw = ropep.tile([P, TT], bf16, name="qsw")
                nc.scalar.copy(qsw[0:64, :], qraw[64:128, :])
                nc.scalar.copy(qsw[64:128, :], qraw[0:64, :])
                p1 = ropep.tile([P, TT], bf16, name="p1")
                nc.vector.tensor_tensor(
                    out=p1[:], in0=qraw[:], in1=cos_sb[:, tsl], op=MUL)
                p2 = ropep.tile([P, TT], bf16, name="p2")
                nc.vector.tensor_tensor(
                    out=p2[:], in0=qsw[:], in1=sin_sb[:, tsl], op=MUL)
                nc.vector.tensor_tensor(
                    out=dst[:, fc, tsl], in0=p1[:], in1=p2[:], op=SUB)

            # x-tile prefetch: DMAs ride the sync HWDGE FIFO, but each tile's
            # dma_start is emitted one step ahead of its consumer so it never
            # queues behind the late-firing output stores popped in between.
            xt_tiles = {}

            def prefetch_x(b, tt):
                gt = b * NTT + tt
                xt = xp.tile([P, ND, TT], bf16, name="xt")
                if b == 0 and tt == 0:
                    # loaded in 4 slices so the first q-pass starts at 512KB
                    for g in range(4):
                        nc.sync.dma_start(
                            out=xt[:, 4 * g:4 * g + 4, :],
                            in_=xTb[:, gt, 4 * g:4 * g + 4, :])
                else:
                    nc.sync.dma_start(out=xt[:], in_=xTb[:, gt, :, :])
                xt_tiles[(b, tt)] = xt

            def emit_qkv(b, tt):
                tsl = slice(tt * TT, (tt + 1) * TT)
                # issue the NEXT tile's x load now (runs during this tile) —
                # except on the very first tile, where it would steal DMA
                # bandwidth from the critical wq/wk/wv loads; there it is
                # deferred to the v-pass (still ~15us ahead of its use).
                nxt = (b, tt + 1) if tt + 1 < NTT else (b + 1, 0)
                first = (b == 0 and tt == 0)
                if nxt[0] < B and not first:
                    prefetch_x(*nxt)
                xt = xt_tiles.pop((b, tt))

                # q/k passes: one PSUM bank per (which, fc), sequential
                for w_sb, dst in ((wq_sb, qT_sb), (wk_sb, kT_sb)):
                    for fc in range(2):
                        ps = ps_qs.tile([P, TT], f32, name="qs")
                        for d in range(ND):
                            nc.tensor.matmul(
                                ps[:],
                                w_sb[:, d, fc * P:(fc + 1) * P],
                                xt[:, d, :],
                                start=(d == 0), stop=(d == ND - 1),
                            )
                        rope_evict(ps, dst[:, b], fc, tsl)
                        pop_wo()
                if first:
                    prefetch_x(*nxt)
                # v pass: natural layout, one 128-token sub-chunk per bank
                for sub in range(4):
                    ps = ps_qs.tile([P, TT], f32, name="qs")
                    for d in range(ND):
                        nc.tensor.matmul(
                            ps[:, 0:FPC],
                            xt[:, d, sub * P:(sub + 1) * P],
                            wv_sb[:, d, :],
                            start=(d == 0), stop=(d == ND - 1),
                        )
                    nc.any.tensor_copy(
                        v_sb[:, b, tt * 4 + sub, :], ps[:, 0:FPC])
                    pop_wo()

            def emit_attention(b, qt):
                t0g = b * S
                nkt = 4 * qt + 4
                aT = aTp.tile([P, 2, TT], bf16, name="aT")
                ost = ostp.tile([P, 4, D], bf16, name="ost")
                ost_state[id(ost)] = [0, b, qt]
                # spread pending wo jobs over this tile's pair iterations
                niter = nkt
                wo_quota = len(wo_jobs)
                emitted = [0]

                def pace(i):
                    want = wo_quota * (i + 1) // niter
                    while emitted[0] < want and wo_jobs:
                        emit_wo_group(*wo_jobs.pop(0))
                        emitted[0] += 1

                it = 0
                for h in range(2):
                    acc = accp.tile([P, TT], bf16, name="acc")
                    a_ps = ps_a.tile([P, TT], f32, name="aps")
                    for kt in range(nkt):
                        if kt % 2 == 0:
                            pace(it); it += 1
                        o = kt - 4 * qt
                        c0 = 0 if o < 0 else o * P
                        csl = slice(c0, TT)
                        s_ps = ps_qs.tile([P, TT], f32, name="qs")
                        nc.tensor.matmul(
                            s_ps[:, csl],
                            kT_sb[:, b, h, kt * P:(kt + 1) * P],
                            qT_sb[:, b, h, qt * TT + c0:(qt + 1) * TT],
                            start=True, stop=True,
                        )
                        pr = probsp.tile([P, TT], bf16, name="probs")
                        nc.scalar.activation(
                            pr[:, csl], s_ps[:, csl], EXP, scale=SCALE)
                        if o >= 0:
                            nc.vector.tensor_tensor(
                                out=pr[:, c0:c0 + P], in0=pr[:, c0:c0 + P],
                                in1=mask_sb[:, P:2 * P], op=MUL)
                        if kt == 0:
                            nc.vector.tensor_copy(acc[:], pr[:])
                        else:
                            nc.vector.tensor_tensor(
                                out=acc[:, csl], in0=acc[:, csl],
                                in1=pr[:, csl], op=ADD)
                        nc.tensor.matmul(
                            a_ps[:, csl],
                            v_sb[:, b, kt, h * P:(h + 1) * P],
                            pr[:, csl],
                            start=(kt == 0), stop=(kt == nkt - 1),
                        )
                    # softmax denominator: ones-matmul broadcasts the
                    # partition-sum of acc to all 128 partitions in one
                    # 512-cycle PE op; ~1.5us chain latency to aT.
                    d_ps = ps_qs.tile([P, TT], f32, name="qs")
                    nc.tensor.matmul(
                        d_ps[:], ones_sb[:], acc[:], start=True, stop=True)
                    rb = dp.tile([P, TT], f32, name="rb")
                    nc.vector.reciprocal_approx_fast(rb[:], d_ps[:])
                    nc.vector.tensor_tensor(
                        out=aT[:, h, :], in0=a_ps[:], in1=rb[:], op=MUL)
                for ts in range(4):
                    for jc in range(D // TT):
                        wo_jobs.append((b, qt, aT, ts, jc, ost))

            # schedule: all QKV tiles of a batch, then all its attention
            # tiles — the exp-heavy attention phases then overlap the next
            # batch's rope-light QKV, smoothing the ACT/DVE load.
            prefetch_x(0, 0)
            for b in range(B):
                for tt in range(NTT):
                    emit_qkv(b, tt)
                for qt in range(NTT):
                    emit_attention(b, qt)
            pop_wo(len(wo_jobs))
    nc.compile()
    return nc


def _host_prep(x, wq, wk, wv, wo):
    import ml_dtypes

    bf = ml_dtypes.bfloat16
    x = np.asarray(x, dtype=np.float32)
    wq = np.asarray(wq, dtype=np.float32)
    wk = np.asarray(wk, dtype=np.float32)
    wv = np.asarray(wv, dtype=np.float32)
    wo = np.asarray(wo, dtype=np.float32)

    # x pre-tiled: [P, global token tile, d-chunk, token] with contiguous
    # per-partition runs per (tile, d-chunk)
    xT = x.reshape(B * S, D).T                        # [D, B*S]
    xTt = np.ascontiguousarray(
        xT.reshape(ND, P, B * NTT, TT).transpose(1, 2, 0, 3)).astype(bf)

    def tile_w(w):  # [D, FPC] -> [P, ND, FPC]
        return np.ascontiguousarray(
            w.reshape(ND, P, FPC).transpose(1, 0, 2)).astype(bf)

    # permute q/k head dims: per head, even dims then odd dims
    perm = np.concatenate(
        [h * HD + np.concatenate([np.arange(0, HD, 2), np.arange(1, HD, 2)])
         for h in range(H)]
    )
    wq_p = wq[perm]
    wk_p = wk[perm]

    # rope tables; cos stacked twice, sin stacked [s; -s]
    inv_freq = 1.0 / (10000.0 ** (np.arange(0, HD, 2, dtype=np.float64) / HD))
    t = np.arange(S, dtype=np.float64)
    freqs = t[:, None] * inv_freq[None, :]            # [S, 64]
    cosT = np.cos(freqs).T.astype(np.float32)         # [64, S]
    sinT = np.sin(freqs).T.astype(np.float32)
    cosS = np.ascontiguousarray(np.vstack([cosT, cosT])).astype(bf)
    sinS = np.ascontiguousarray(np.vstack([sinT, -sinT])).astype(bf)

    # masks: [zeros(128) | lower-triangular(128)] for the diagonal blocks
    pidx = np.arange(P)[:, None]
    qidx = np.arange(P)[None, :]
    tri = (qidx >= pidx).astype(np.float32)
    m = np.ascontiguousarray(
        np.hstack([np.zeros((P, P), np.float32), tri])).astype(bf)
    ones = np.ones((P, P), dtype=np.float32).astype(bf)

    in_maps = []
    for c in range(NCORES):
        fs = slice(c * FPC, (c + 1) * FPC)
        woc = wo[:, fs].T                              # [256, D]
        in_maps.append({
            "xTb": xTt,
            "wqT": tile_w(wq_p[fs].T),                 # [P, ND, FPC]
            "wkT": tile_w(wk_p[fs].T),
            "wvT": tile_w(wv[fs].T),
            "woT": np.ascontiguousarray(
                woc.reshape(2, P, D).transpose(1, 0, 2)).astype(bf),
            "cosS": cosS,
            "sinS": sinS,
            "masks": m,
            "onesd": ones,
        })
    return in_maps


def _run(inputs, trace=False):
    from concourse.bass_utils import run_bass_kernel_spmd

    if "nc" not in _CACHE:
        _CACHE["nc"] = _build_nc()
    nc = _CACHE["nc"]

    in_maps = _host_prep(
        inputs["x"], inputs["wq"], inputs["wk"], inputs["wv"], inputs["wo"]
    )
    res = run_bass_kernel_spmd(nc, in_maps, list(range(NCORES)), trace=trace)
    acc = None
    for c in range(NCORES):
        part = res.results[c]["outp"].astype(np.float32)
        acc = part if acc is None else acc + part
    out = acc.reshape(B, S, D).astype(np.float32)
    return out, res


def kernel(**inputs) -> np.ndarray:
    out, _ = _run(inputs, trace=False)
    return out
